# revision 38
# baseline (speedup 1.0000x reference)
"""Batch-all triplet loss on 8 Trainium2 NeuronCores.

Math (reference): d = pairwise euclidean distances of embeddings [512, 256];
tl[i,j,k] = relu((d_ij - d_ik + margin) * mask); loss = sum(tl) / (count(tl > eps) + eps)
where mask requires labels[i]==labels[j], labels[i]!=labels[k], i!=j (j!=k, i!=k follow).

Device formulation: A[i,j] = d_ij + margin where (labels equal, i!=j) else -BIG;
B[i,k] = -d_ik where labels differ else -BIG.  Then per (i,j,k):
v = relu(A[i,j] + B[i,k]) equals the masked triplet loss exactly (masked
entries produce relu(very negative) = 0), and count = #{v > eps}.

Sharding: anchor axis i split 64-per-core across 8 cores.  Each core computes
its [64, 512, 512] block as 64x4 tiles of [128 j, 512 k]:
  pass1: v = max(B_bcast + A_col, 0)   (+ fused per-partition row-sum)
  pass2: is_gt(v, eps)                 (+ fused per-partition count)
pass1 runs on ScalarE for 2 of 4 j-chunks and VectorE for the rest; pass2 on
VectorE (bf16 tiles -> 4x DVE mode).  Partial sums [128, 3] per core are
reduced on host: loss = S / (C + eps).
"""

import contextlib
import ctypes
import sys
import types

import numpy as np

import bass_rust
import concourse.bass as bass
import concourse.tile as tile
from concourse import mybir
from concourse.bass_utils import run_bass_kernel_spmd
from concourse.vector_clock import ScopedClock

# ---------------------------------------------------------------------------
# Environment shims (walrus drain-wait limit + NTFF profile hook under axon)
# ---------------------------------------------------------------------------

_MAX_WAITS_PER_INST = 1
_AXON_SO_PATH = "/opt/axon/libaxon_pjrt.so"


def _patched_drain_and_barrier(self, tick_clock, wait_clock):
    nc = self.nc
    drain_inst = nc.sync.drain()
    wait_clock.add_sem_waits(
        drain_inst.ins, ScopedClock({None: tick_clock.global_clock})
    )
    si = drain_inst.ins.sync_info
    if si is not None and si.on_wait and len(si.on_wait) > _MAX_WAITS_PER_INST:
        waits = list(si.on_wait)
        si.on_wait = waits[:_MAX_WAITS_PER_INST]
        rest = waits[_MAX_WAITS_PER_INST:]
        for i in range(0, len(rest), _MAX_WAITS_PER_INST):
            extra = nc.sync.drain()
            extra.ins.sync_info = bass_rust.SyncInfo(
                on_wait=rest[i : i + _MAX_WAITS_PER_INST], on_update=[]
            )

    nc.all_engine_barrier()
    assert self.sems is not None
    popped = nc._tile_sem_poison_stack.pop()
    assert popped is self._sem_poison
    nc.clear_and_free_semaphores(list(self.sems.allocated().values()))
    nc.all_engine_barrier()


def _ntff_profile_via_ctypes(so_path):
    try:
        lib = ctypes.CDLL(so_path)
    except OSError:
        return None
    if not hasattr(lib, "axon_start_nrt_profile"):
        return None
    lib.axon_start_nrt_profile.argtypes = [
        ctypes.POINTER(ctypes.c_int64),
        ctypes.c_size_t,
    ]
    lib.axon_start_nrt_profile.restype = ctypes.c_int64
    lib.axon_stop_nrt_profile.argtypes = [ctypes.c_char_p]
    lib.axon_stop_nrt_profile.restype = ctypes.c_int64

    @contextlib.contextmanager
    def _hook(output_dir, device_ids):
        import jax

        jax.devices()
        if device_ids:
            ids = (ctypes.c_int64 * len(device_ids))(*device_ids)
            rc = lib.axon_start_nrt_profile(ids, len(device_ids))
        else:
            rc = lib.axon_start_nrt_profile(None, 0)
        if rc != 0:
            raise RuntimeError(f"axon_start_nrt_profile rc={rc}")
        try:
            yield
        finally:
            n = lib.axon_stop_nrt_profile(str(output_dir).encode())
            if n < 0:
                raise RuntimeError(f"axon_stop_nrt_profile rc={n}")
            if n == 0:
                print(f"profile: ZERO files written to {output_dir}", file=sys.stderr)

    return _hook


def _split_sync_waits(nc, max_waits=1):
    """This toolchain's walrus rejects instructions carrying more than one
    semaphore wait.  Hoist extra waits onto standalone EventSemaphore
    instructions inserted just before the owner (same engine, so sequencer
    program order preserves the blocking semantics)."""
    n_split = 0
    for f in nc.m.functions:
        for blk in f.blocks:
            insts = list(blk.instructions)
            out = []
            changed = False
            for inst in insts:
                si = getattr(inst, "sync_info", None)
                if si is not None and si.on_wait and len(si.on_wait) > max_waits:
                    waits = list(si.on_wait)
                    extra, keep = waits[:-max_waits], waits[-max_waits:]
                    for j, w in enumerate(extra):
                        ev = mybir.InstEventSemaphore(name=f"{inst.name}-sw{j}", ins=[], outs=[])
                        ev.name = f"{inst.name}-sw{j}"
                        ev.engine = inst.engine
                        ev.sync_info = bass_rust.SyncInfo(on_wait=[w], on_update=[])
                        out.append(ev)
                        n_split += 1
                    si.on_wait = keep
                    changed = True
                out.append(inst)
            if changed:
                blk.instructions = out
    return n_split


def _install_shims():
    tile.TileContext._drain_and_barrier = _patched_drain_and_barrier
    if "antenv.axon_hooks" not in sys.modules:
        m = types.ModuleType("antenv.axon_hooks")
        _state = {"hook": _ntff_profile_via_ctypes(_AXON_SO_PATH)}
        m.set_axon_ntff_profile_hook = lambda hook: _state.__setitem__("hook", hook)
        m.get_axon_ntff_profile_hook = lambda: _state["hook"]
        sys.modules["antenv.axon_hooks"] = m


_install_shims()

# ---------------------------------------------------------------------------
# Problem constants (hardcoded per spec)
# ---------------------------------------------------------------------------

B = 512          # batch (anchors)
D = 256          # embedding dim
N_CORES = 8
PA = B // N_CORES  # anchors per core = 64
EPS = 1e-8
MARGIN = 1.0
BIG = 50000.0
N_JCHUNK = B // 128  # 4 j-chunks of 128 partitions
# count engine assignment over anchor PAIRS: SIGN_NUM of every SIGN_DEN pairs
# count via one batched Sign activation on ScalarE (over both anchors' v
# tiles); the rest via batched is_gt on VectorE plus TensorE ones-matmuls.
SIGN_NUM = 3
SIGN_DEN = 5
GROUP = 1  # anchors per count group
DSHIFT = 0.5  # added under the sqrt; keeps it clamp-free, cancels in d_ij-d_ik

f32 = mybir.dt.float32
bf16 = mybir.dt.bfloat16
Alu = mybir.AluOpType
Act = mybir.ActivationFunctionType


def _build_program():
    nc = bass.Bass("TRN2", target_bir_lowering=False, debug=False)

    xT = nc.dram_tensor("xT", [D, B], f32, kind="ExternalInput").ap()
    labf = nc.dram_tensor("labf", [B], f32, kind="ExternalInput").ap()
    eyem = nc.dram_tensor("eyem", [PA, B], f32, kind="ExternalInput").ap()
    ident = nc.dram_tensor("ident", [64, 64], f32, kind="ExternalInput").ap()
    out = nc.dram_tensor("out", [128, 2], f32, kind="ExternalOutput").ap()
    out2 = nc.dram_tensor("out2", [1, 2 * B], f32, kind="ExternalOutput").ap()

    b_dram = nc.dram_tensor("b_scratch", [PA, B], bf16, kind="Internal").ap()

    # The per-core anchor window is expressed through input VALUES (one SPMD
    # program): the host passes core-sliced tensors directly.
    xTo = nc.dram_tensor("xT_ours", [D, PA], f32, kind="ExternalInput").ap()
    labo = nc.dram_tensor("lab_ours", [PA], f32, kind="ExternalInput").ap()
    xo_in = nc.dram_tensor("x_ours", [PA, D], f32, kind="ExternalInput").ap()

    with tile.TileContext(nc) as tc:
        with contextlib.ExitStack() as ctx:
            singles = ctx.enter_context(tc.tile_pool(name="singles", bufs=1))
            prol = ctx.enter_context(tc.tile_pool(name="prol", bufs=2))
            psum_p = ctx.enter_context(tc.tile_pool(name="psum_p", bufs=1, space="PSUM"))
            psum_t = ctx.enter_context(tc.tile_pool(name="psum_t", bufs=2, space="PSUM"))
            bcast = ctx.enter_context(tc.tile_pool(name="bcast", bufs=6))
            vpool = ctx.enter_context(tc.tile_pool(name="vpool", bufs=4))
            scrp = ctx.enter_context(tc.tile_pool(name="scrp", bufs=3))

            # ---- loads (spread across two DMA paths) -------------------
            xT_t = singles.tile([128, 2, B], f32)          # xT[(c p), k] -> [p, c, k]
            nc.sync.dma_start(out=xT_t, in_=xT.rearrange("(c p) k -> p c k", p=128))
            xTo_t = singles.tile([128, 2, PA], f32)        # our anchor columns
            nc.gpsimd.dma_start(out=xTo_t, in_=xTo.rearrange("(c p) k -> p c k", p=128))
            labo_t = singles.tile([PA, 1], f32)
            nc.gpsimd.dma_start(
                out=labo_t,
                in_=bass.AP(tensor=labo.tensor, offset=labo.offset, ap=[[1, PA], [1, 1]]),
            )
            lab_bcast = singles.tile([PA, B], f32)
            nc.gpsimd.dma_start(
                out=lab_bcast,
                in_=bass.AP(tensor=labf.tensor, offset=labf.offset, ap=[[0, PA]] + list(labf.ap)),
            )
            eyem_t = singles.tile([PA, B], f32)
            nc.gpsimd.dma_start(out=eyem_t, in_=eyem)
            ident_t = singles.tile([64, 64], f32)
            nc.gpsimd.dma_start(out=ident_t, in_=ident)
            xo_t = singles.tile([PA, D], f32)
            nc.sync.dma_start(out=xo_t, in_=xo_in)

            ones_bf = singles.tile([128, 1], bf16)
            nc.vector.memset(ones_bf, 1.0)
            epsb = singles.tile([128, 1], f32)
            nc.vector.memset(epsb, -EPS)

            # ---- sq_k = sum_d x^2 (PE path: square then ones-matmul) ---
            xsq = singles.tile([128, 2, B], bf16)
            nc.vector.scalar_tensor_tensor(
                out=xsq, in0=xT_t, scalar=1.0, in1=xT_t,
                op0=Alu.mult, op1=Alu.mult,
            )
            psum_sq = psum_p.tile([1, B], f32, tag="psum_sq")
            nc.tensor.matmul(psum_sq, ones_bf, xsq[:, 0, :], start=True, stop=False)
            nc.tensor.matmul(psum_sq, ones_bf, xsq[:, 1, :], start=False, stop=True)
            sq_row = singles.tile([1, B], f32)
            nc.vector.tensor_copy(sq_row, psum_sq)
            # our anchors' sq as a [PA, 1] per-partition scalar (from x_ours)
            sqo_t = singles.tile([PA, 1], f32)
            sqo_scratch = prol.tile([PA, D], f32, tag="sqo_scratch")
            nc.vector.scalar_tensor_tensor(
                out=sqo_scratch, in0=xo_t, scalar=1.0, in1=xo_t,
                op0=Alu.mult, op1=Alu.mult, accum_out=sqo_t,
            )
            # +DSHIFT so sqrt's argument stays positive without a clamp op
            # (fp residue on the diagonal can be slightly negative).  The
            # distance shift cancels to first order in d_ij - d_ik.
            sqo1_t = singles.tile([PA, 1], f32)
            nc.vector.tensor_scalar(
                out=sqo1_t, in0=sqo_t, scalar1=DSHIFT, scalar2=None, op0=Alu.add,
            )

            # ---- G slab: psum_d = -2 * x_ours @ x^T + sq_k -------------
            m2xT = singles.tile([128, 2, PA], f32)
            for c in range(2):
                nc.vector.tensor_scalar(
                    out=m2xT[:, c, :], in0=xTo_t[:, c, :],
                    scalar1=-2.0, scalar2=None, op0=Alu.mult,
                )
            ones1 = singles.tile([1, PA], f32)
            nc.vector.memset(ones1, 1.0)

            psum_d = psum_p.tile([PA, B], f32, tag="psum_d")
            nc.tensor.matmul(psum_d, m2xT[:, 0, :], xT_t[:, 0, :], start=True, stop=False)
            nc.tensor.matmul(psum_d, m2xT[:, 1, :], xT_t[:, 1, :], start=False, stop=False)
            nc.tensor.matmul(psum_d, ones1, sq_row, start=False, stop=True)

            # ---- distances: d = sqrt(psum + sq_i + DSHIFT) -------------
            dslab = singles.tile([PA, B], f32)
            nc.scalar.activation(
                out=dslab, in_=psum_d, func=Act.Sqrt, bias=sqo1_t, scale=1.0,
            )

            # ---- masks and A/B matrices --------------------------------
            leq = prol.tile([PA, B], f32, tag="leq")
            nc.vector.tensor_scalar(
                out=leq, in0=lab_bcast, scalar1=labo_t, scalar2=None, op0=Alu.is_equal,
            )
            a32 = singles.tile([PA, B], f32)
            nc.vector.scalar_tensor_tensor(
                out=a32, in0=leq, scalar=BIG, in1=eyem_t,
                op0=Alu.mult, op1=Alu.subtract,
            )
            nc.vector.tensor_tensor(out=a32, in0=a32, in1=dslab, op=Alu.add)
            bbf = singles.tile([PA, B], bf16)
            nc.vector.scalar_tensor_tensor(
                out=bbf, in0=leq, scalar=-BIG, in1=dslab,
                op0=Alu.mult, op1=Alu.subtract,
            )
            nc.sync.dma_start(out=b_dram, in_=bbf)

            # ---- A transposed: AT[j, c, i] = A[i, c*128 + j] ------------
            at = singles.tile([128, N_JCHUNK, PA], f32)
            for c in range(N_JCHUNK):
                pt = psum_t.tile([128, PA], f32, tag="pt")
                nc.tensor.transpose(pt, a32[:, c * 128 : (c + 1) * 128], ident_t)
                nc.vector.tensor_copy(at[:, c, :], pt)

            # ---- accumulators ------------------------------------------
            n_tiles = PA * N_JCHUNK
            n_groups = PA // GROUP
            sign_groups = [g for g in range(n_groups) if g % SIGN_DEN < SIGN_NUM]
            n_sign = len(sign_groups)
            n_dvec = n_groups - n_sign
            csA = singles.tile([128, max(n_sign, 1)], f32)
            psum_vs = psum_p.tile([1, B], f32, tag="psum_vs")
            psum_cs = psum_p.tile([1, B], f32, tag="psum_cs")

            # ---- main loop ---------------------------------------------
            # pass1: all chunks on DVE (fused add+relu, bf16 4x) into one
            # per-group [128, GROUP*4, B] tile; v row-sums via TensorE
            # ones-matmuls.  count: per anchor-group either one batched Sign
            # activation on ScalarE (count = (sum(sign(v-eps)) + N)/2) or a
            # batched is_gt on DVE plus TensorE ones-matmuls.
            n_pe_v = 0
            n_pe_c = 0
            n_a = 0
            for g in range(n_groups):
                vg = vpool.tile([128, GROUP, N_JCHUNK, B], bf16, tag="vg")
                for gi in range(GROUP):
                    i = g * GROUP + gi
                    bb = bcast.tile([128, B], bf16, tag="bb")
                    nc.sync.dma_start(
                        out=bb,
                        in_=bass.AP(
                            tensor=b_dram.tensor,
                            offset=b_dram.offset + i * B,
                            ap=[[0, 128], [1, B]],
                        ),
                    )
                    for c in range(N_JCHUNK):
                        bias_ap = at[:, c, i : i + 1]
                        nc.vector.tensor_scalar(
                            out=vg[:, gi, c, :], in0=bb, scalar1=bias_ap,
                            scalar2=0.0, op0=Alu.add, op1=Alu.max,
                        )
                        nc.tensor.matmul(
                            psum_vs, ones_bf, vg[:, gi, c, :],
                            start=(n_pe_v == 0), stop=(n_pe_v == n_tiles - 1),
                        )
                        n_pe_v += 1
                if g % SIGN_DEN < SIGN_NUM:
                    sg = scrp.tile([128, GROUP, N_JCHUNK, B], bf16, tag="sg")
                    nc.scalar.activation(
                        out=sg, in_=vg, func=Act.Sign, bias=epsb, scale=1.0,
                        accum_out=csA[:, n_a : n_a + 1],
                    )
                    n_a += 1
                else:
                    ind4 = scrp.tile([128, GROUP, N_JCHUNK, B], bf16, tag="sg")
                    nc.vector.tensor_scalar(
                        out=ind4, in0=vg, scalar1=EPS, scalar2=None, op0=Alu.is_gt,
                    )
                    for gi in range(GROUP):
                        for c in range(N_JCHUNK):
                            nc.tensor.matmul(
                                psum_cs, ones_bf, ind4[:, gi, c, :],
                                start=(n_pe_c == 0),
                                stop=(n_pe_c == GROUP * N_JCHUNK * n_dvec - 1),
                            )
                            n_pe_c += 1

            # ---- final reduction -> out [128,2], out2 [1,2B] -----------
            out_sb = singles.tile([128, 2], f32)
            nc.vector.reduce_sum(out_sb[:, 0:1], csA, axis=mybir.AxisListType.X)
            nc.vector.memset(out_sb[:, 1:2], 0.0)
            nc.sync.dma_start(out=out, in_=out_sb)
            vs_row = singles.tile([1, 2 * B], f32)
            nc.vector.tensor_copy(vs_row[:, 0:B], psum_vs)
            nc.vector.tensor_copy(vs_row[:, B : 2 * B], psum_cs)
            nc.sync.dma_start(out=out2, in_=vs_row)

    _split_sync_waits(nc)
    return nc


_PROGRAM = None


def _get_program():
    global _PROGRAM
    if _PROGRAM is None:
        _PROGRAM = _build_program()
    return _PROGRAM


def _make_in_maps(x, labels):
    x = np.ascontiguousarray(x, dtype=np.float32)
    labf = np.ascontiguousarray(labels.astype(np.float32))
    xT = np.ascontiguousarray(x.T)
    ident = np.eye(64, dtype=np.float32)
    in_maps = []
    for r in range(N_CORES):
        r0 = r * PA
        eyem = np.full((PA, B), BIG - MARGIN, dtype=np.float32)
        eyem[np.arange(PA), r0 + np.arange(PA)] += BIG
        in_maps.append(
            {
                "xT": xT,
                "labf": labf,
                "eyem": eyem,
                "ident": ident,
                "xT_ours": np.ascontiguousarray(xT[:, r0 : r0 + PA]),
                "lab_ours": np.ascontiguousarray(labf[r0 : r0 + PA]),
                "x_ours": np.ascontiguousarray(x[r0 : r0 + PA, :]),
            }
        )
    return in_maps


def run_device(x, labels, trace=False):
    """Run the SPMD program; returns (results, exec_time_ns)."""
    nc = _get_program()
    res = run_bass_kernel_spmd(
        nc, _make_in_maps(x, labels), core_ids=list(range(N_CORES)), trace=trace
    )
    return res


def host_reduce(res):
    n_groups = PA // GROUP
    n_sign = len([g for g in range(n_groups) if g % SIGN_DEN < SIGN_NUM])
    sign_elems = n_sign * 128 * GROUP * N_JCHUNK * B  # +/-1 terms per core
    S = 0.0
    C = 0.0
    for r in range(N_CORES):
        o = res.results[r]["out"].astype(np.float64)
        o2 = res.results[r]["out2"].astype(np.float64).reshape(2, B)
        S += o2[0].sum()
        C += o2[1].sum() + (o[:, 0].sum() + sign_elems) / 2.0
    return np.float32(S / (C + EPS))


def kernel(embeddings, labels):
    res = run_device(np.asarray(embeddings), np.asarray(labels), trace=False)
    return host_reduce(res)


# revision 44
# speedup vs baseline: 1.0056x; 1.0056x over previous
"""Batch-all triplet loss on 8 Trainium2 NeuronCores.

Math (reference): d = pairwise euclidean distances of embeddings [512, 256];
tl[i,j,k] = relu((d_ij - d_ik + margin) * mask); loss = sum(tl) / (count(tl > eps) + eps)
where mask requires labels[i]==labels[j], labels[i]!=labels[k], i!=j (j!=k, i!=k follow).

Device formulation: A[i,j] = d_ij + margin where (labels equal, i!=j) else -BIG;
B[i,k] = -d_ik where labels differ else -BIG.  Then per (i,j,k):
v = relu(A[i,j] + B[i,k]) equals the masked triplet loss exactly (masked
entries produce relu(very negative) = 0), and count = #{v > eps}.

Sharding: anchor axis i split 64-per-core across 8 cores.  Each core computes
its [64, 512, 512] block as 64x4 tiles of [128 j, 512 k]:
  pass1: v = max(B_bcast + A_col, 0)   (+ fused per-partition row-sum)
  pass2: is_gt(v, eps)                 (+ fused per-partition count)
pass1 runs on ScalarE for 2 of 4 j-chunks and VectorE for the rest; pass2 on
VectorE (bf16 tiles -> 4x DVE mode).  Partial sums [128, 3] per core are
reduced on host: loss = S / (C + eps).
"""

import contextlib
import ctypes
import sys
import types

import numpy as np

import bass_rust
import concourse.bass as bass
import concourse.tile as tile
from concourse import mybir
from concourse.bass_utils import run_bass_kernel_spmd
from concourse.vector_clock import ScopedClock

# ---------------------------------------------------------------------------
# Environment shims (walrus drain-wait limit + NTFF profile hook under axon)
# ---------------------------------------------------------------------------

_MAX_WAITS_PER_INST = 1
_AXON_SO_PATH = "/opt/axon/libaxon_pjrt.so"


def _patched_drain_and_barrier(self, tick_clock, wait_clock):
    nc = self.nc
    drain_inst = nc.sync.drain()
    wait_clock.add_sem_waits(
        drain_inst.ins, ScopedClock({None: tick_clock.global_clock})
    )
    si = drain_inst.ins.sync_info
    if si is not None and si.on_wait and len(si.on_wait) > _MAX_WAITS_PER_INST:
        waits = list(si.on_wait)
        si.on_wait = waits[:_MAX_WAITS_PER_INST]
        rest = waits[_MAX_WAITS_PER_INST:]
        for i in range(0, len(rest), _MAX_WAITS_PER_INST):
            extra = nc.sync.drain()
            extra.ins.sync_info = bass_rust.SyncInfo(
                on_wait=rest[i : i + _MAX_WAITS_PER_INST], on_update=[]
            )

    nc.all_engine_barrier()
    assert self.sems is not None
    popped = nc._tile_sem_poison_stack.pop()
    assert popped is self._sem_poison
    nc.clear_and_free_semaphores(list(self.sems.allocated().values()))
    nc.all_engine_barrier()


def _ntff_profile_via_ctypes(so_path):
    try:
        lib = ctypes.CDLL(so_path)
    except OSError:
        return None
    if not hasattr(lib, "axon_start_nrt_profile"):
        return None
    lib.axon_start_nrt_profile.argtypes = [
        ctypes.POINTER(ctypes.c_int64),
        ctypes.c_size_t,
    ]
    lib.axon_start_nrt_profile.restype = ctypes.c_int64
    lib.axon_stop_nrt_profile.argtypes = [ctypes.c_char_p]
    lib.axon_stop_nrt_profile.restype = ctypes.c_int64

    @contextlib.contextmanager
    def _hook(output_dir, device_ids):
        import jax

        jax.devices()
        if device_ids:
            ids = (ctypes.c_int64 * len(device_ids))(*device_ids)
            rc = lib.axon_start_nrt_profile(ids, len(device_ids))
        else:
            rc = lib.axon_start_nrt_profile(None, 0)
        if rc != 0:
            raise RuntimeError(f"axon_start_nrt_profile rc={rc}")
        try:
            yield
        finally:
            n = lib.axon_stop_nrt_profile(str(output_dir).encode())
            if n < 0:
                raise RuntimeError(f"axon_stop_nrt_profile rc={n}")
            if n == 0:
                print(f"profile: ZERO files written to {output_dir}", file=sys.stderr)

    return _hook


def _split_sync_waits(nc, max_waits=1):
    """This toolchain's walrus rejects instructions carrying more than one
    semaphore wait.  Hoist extra waits onto standalone EventSemaphore
    instructions inserted just before the owner (same engine, so sequencer
    program order preserves the blocking semantics)."""
    n_split = 0
    for f in nc.m.functions:
        for blk in f.blocks:
            insts = list(blk.instructions)
            out = []
            changed = False
            for inst in insts:
                si = getattr(inst, "sync_info", None)
                if si is not None and si.on_wait and len(si.on_wait) > max_waits:
                    waits = list(si.on_wait)
                    extra, keep = waits[:-max_waits], waits[-max_waits:]
                    for j, w in enumerate(extra):
                        ev = mybir.InstEventSemaphore(name=f"{inst.name}-sw{j}", ins=[], outs=[])
                        ev.name = f"{inst.name}-sw{j}"
                        ev.engine = inst.engine
                        ev.sync_info = bass_rust.SyncInfo(on_wait=[w], on_update=[])
                        out.append(ev)
                        n_split += 1
                    si.on_wait = keep
                    changed = True
                out.append(inst)
            if changed:
                blk.instructions = out
    return n_split


def _install_shims():
    tile.TileContext._drain_and_barrier = _patched_drain_and_barrier
    if "antenv.axon_hooks" not in sys.modules:
        m = types.ModuleType("antenv.axon_hooks")
        _state = {"hook": _ntff_profile_via_ctypes(_AXON_SO_PATH)}
        m.set_axon_ntff_profile_hook = lambda hook: _state.__setitem__("hook", hook)
        m.get_axon_ntff_profile_hook = lambda: _state["hook"]
        sys.modules["antenv.axon_hooks"] = m


_install_shims()

# ---------------------------------------------------------------------------
# Problem constants (hardcoded per spec)
# ---------------------------------------------------------------------------

B = 512          # batch (anchors)
D = 256          # embedding dim
N_CORES = 8
PA = B // N_CORES  # anchors per core = 64
EPS = 1e-8
MARGIN = 1.0
BIG = 50000.0
N_JCHUNK = B // 128  # 4 j-chunks of 128 partitions
# count engine assignment over anchor PAIRS: SIGN_NUM of every SIGN_DEN pairs
# count via one batched Sign activation on ScalarE (over both anchors' v
# tiles); the rest via batched is_gt on VectorE plus TensorE ones-matmuls.
SIGN_NUM = 3
SIGN_DEN = 5
GROUP = 1  # anchors per count group
DSHIFT = 0.5  # added under the sqrt; keeps it clamp-free, cancels in d_ij-d_ik

f32 = mybir.dt.float32
bf16 = mybir.dt.bfloat16
Alu = mybir.AluOpType
Act = mybir.ActivationFunctionType


def _build_program():
    nc = bass.Bass("TRN2", target_bir_lowering=False, debug=False)

    xT = nc.dram_tensor("xT", [D, B], f32, kind="ExternalInput").ap()
    labf = nc.dram_tensor("labf", [B], f32, kind="ExternalInput").ap()
    eyem = nc.dram_tensor("eyem", [PA, B], f32, kind="ExternalInput").ap()
    ident = nc.dram_tensor("ident", [64, 64], f32, kind="ExternalInput").ap()
    out = nc.dram_tensor("out", [128, 2], f32, kind="ExternalOutput").ap()
    out2 = nc.dram_tensor("out2", [8, B], f32, kind="ExternalOutput").ap()

    b_dram = nc.dram_tensor("b_scratch", [PA, B], bf16, kind="Internal").ap()

    # The per-core anchor window is expressed through input VALUES (one SPMD
    # program): the host passes core-sliced tensors directly.
    xTo = nc.dram_tensor("xT_ours", [D, PA], f32, kind="ExternalInput").ap()
    labo = nc.dram_tensor("lab_ours", [PA], f32, kind="ExternalInput").ap()
    xo_in = nc.dram_tensor("x_ours", [PA, D], f32, kind="ExternalInput").ap()

    with tile.TileContext(nc) as tc:
        with contextlib.ExitStack() as ctx:
            singles = ctx.enter_context(tc.tile_pool(name="singles", bufs=1))
            prol = ctx.enter_context(tc.tile_pool(name="prol", bufs=2))
            psum_p = ctx.enter_context(tc.tile_pool(name="psum_p", bufs=1, space="PSUM"))
            psum_t = ctx.enter_context(tc.tile_pool(name="psum_t", bufs=2, space="PSUM"))
            bcast = ctx.enter_context(tc.tile_pool(name="bcast", bufs=6))
            vpool = ctx.enter_context(tc.tile_pool(name="vpool", bufs=4))
            scrp = ctx.enter_context(tc.tile_pool(name="scrp", bufs=3))

            # ---- loads (spread across two DMA paths) -------------------
            xT_t = singles.tile([128, 2, B], f32)          # xT[(c p), k] -> [p, c, k]
            nc.sync.dma_start(out=xT_t, in_=xT.rearrange("(c p) k -> p c k", p=128))
            xTo_t = singles.tile([128, 2, PA], f32)        # our anchor columns
            nc.gpsimd.dma_start(out=xTo_t, in_=xTo.rearrange("(c p) k -> p c k", p=128))
            labo_t = singles.tile([PA, 1], f32)
            nc.gpsimd.dma_start(
                out=labo_t,
                in_=bass.AP(tensor=labo.tensor, offset=labo.offset, ap=[[1, PA], [1, 1]]),
            )
            lab_bcast = singles.tile([PA, B], f32)
            nc.gpsimd.dma_start(
                out=lab_bcast,
                in_=bass.AP(tensor=labf.tensor, offset=labf.offset, ap=[[0, PA]] + list(labf.ap)),
            )
            eyem_t = singles.tile([PA, B], f32)
            nc.gpsimd.dma_start(out=eyem_t, in_=eyem)
            ident_t = singles.tile([64, 64], f32)
            nc.gpsimd.dma_start(out=ident_t, in_=ident)
            xo_t = singles.tile([PA, D], f32)
            nc.sync.dma_start(out=xo_t, in_=xo_in)

            ones_bf = singles.tile([128, 1], bf16)
            nc.vector.memset(ones_bf, 1.0)
            epsb = singles.tile([128, 1], f32)
            nc.vector.memset(epsb, -EPS)

            # ---- sq_k = sum_d x^2 (PE path: square then ones-matmul) ---
            xsq = singles.tile([128, 2, B], bf16)
            nc.vector.scalar_tensor_tensor(
                out=xsq, in0=xT_t, scalar=1.0, in1=xT_t,
                op0=Alu.mult, op1=Alu.mult,
            )
            psum_sq = psum_p.tile([1, B], f32, tag="psum_sq")
            nc.tensor.matmul(psum_sq, ones_bf, xsq[:, 0, :], start=True, stop=False)
            nc.tensor.matmul(psum_sq, ones_bf, xsq[:, 1, :], start=False, stop=True)
            sq_row = singles.tile([1, B], f32)
            nc.vector.tensor_copy(sq_row, psum_sq)
            # our anchors' sq as a [PA, 1] per-partition scalar (from x_ours)
            sqo_t = singles.tile([PA, 1], f32)
            sqo_scratch = prol.tile([PA, D], f32, tag="sqo_scratch")
            nc.vector.scalar_tensor_tensor(
                out=sqo_scratch, in0=xo_t, scalar=1.0, in1=xo_t,
                op0=Alu.mult, op1=Alu.mult, accum_out=sqo_t,
            )
            # +DSHIFT so sqrt's argument stays positive without a clamp op
            # (fp residue on the diagonal can be slightly negative).  The
            # distance shift cancels to first order in d_ij - d_ik.
            sqo1_t = singles.tile([PA, 1], f32)
            nc.vector.tensor_scalar(
                out=sqo1_t, in0=sqo_t, scalar1=DSHIFT, scalar2=None, op0=Alu.add,
            )

            # ---- G slab: psum_d = -2 * x_ours @ x^T + sq_k -------------
            m2xT = singles.tile([128, 2, PA], f32)
            for c in range(2):
                nc.vector.tensor_scalar(
                    out=m2xT[:, c, :], in0=xTo_t[:, c, :],
                    scalar1=-2.0, scalar2=None, op0=Alu.mult,
                )
            ones1 = singles.tile([1, PA], f32)
            nc.vector.memset(ones1, 1.0)

            psum_d = psum_p.tile([PA, B], f32, tag="psum_d")
            nc.tensor.matmul(psum_d, m2xT[:, 0, :], xT_t[:, 0, :], start=True, stop=False)
            nc.tensor.matmul(psum_d, m2xT[:, 1, :], xT_t[:, 1, :], start=False, stop=False)
            nc.tensor.matmul(psum_d, ones1, sq_row, start=False, stop=True)

            # ---- distances: d = sqrt(psum + sq_i + DSHIFT) -------------
            dslab = singles.tile([PA, B], f32)
            nc.scalar.activation(
                out=dslab, in_=psum_d, func=Act.Sqrt, bias=sqo1_t, scale=1.0,
            )

            # ---- masks and A/B matrices --------------------------------
            leq = prol.tile([PA, B], f32, tag="leq")
            nc.vector.tensor_scalar(
                out=leq, in0=lab_bcast, scalar1=labo_t, scalar2=None, op0=Alu.is_equal,
            )
            a32 = singles.tile([PA, B], f32)
            nc.vector.scalar_tensor_tensor(
                out=a32, in0=leq, scalar=BIG, in1=eyem_t,
                op0=Alu.mult, op1=Alu.subtract,
            )
            nc.vector.tensor_tensor(out=a32, in0=a32, in1=dslab, op=Alu.add)
            bbf = singles.tile([PA, B], bf16)
            nc.vector.scalar_tensor_tensor(
                out=bbf, in0=leq, scalar=-BIG, in1=dslab,
                op0=Alu.mult, op1=Alu.subtract,
            )
            nc.sync.dma_start(out=b_dram, in_=bbf)

            # ---- A transposed: AT[j, c, i] = A[i, c*128 + j] ------------
            at = singles.tile([128, N_JCHUNK, PA], f32)
            for c in range(N_JCHUNK):
                pt = psum_t.tile([128, PA], f32, tag="pt")
                nc.tensor.transpose(pt, a32[:, c * 128 : (c + 1) * 128], ident_t)
                nc.vector.tensor_copy(at[:, c, :], pt)

            # ---- accumulators ------------------------------------------
            n_tiles = PA * N_JCHUNK
            n_groups = PA // GROUP
            sign_groups = [g for g in range(n_groups) if g % SIGN_DEN < SIGN_NUM]
            n_sign = len(sign_groups)
            n_dvec = n_groups - n_sign
            csA = singles.tile([128, max(n_sign, 1)], f32)
            # 4 accumulator rows per quantity (partitions 0/32/64/96), fed by
            # column-tiled concurrent ones-matmuls (tile_position=(0, 32c)).
            psum_vs = psum_p.tile([128, B], f32, tag="psum_vs")
            psum_cs = psum_p.tile([128, B], f32, tag="psum_cs")

            # ---- main loop ---------------------------------------------
            # pass1: all chunks on DVE (fused add+relu, bf16 4x) into one
            # per-group [128, GROUP*4, B] tile; v row-sums via TensorE
            # ones-matmuls.  count: per anchor-group either one batched Sign
            # activation on ScalarE (count = (sum(sign(v-eps)) + N)/2) or a
            # batched is_gt on DVE plus TensorE ones-matmuls.
            n_pe_v = 0
            n_pe_c = 0
            n_a = 0
            for g in range(n_groups):
                vg = vpool.tile([128, GROUP, N_JCHUNK, B], bf16, tag="vg")
                for gi in range(GROUP):
                    i = g * GROUP + gi
                    bb = bcast.tile([128, B], bf16, tag="bb")
                    nc.sync.dma_start(
                        out=bb,
                        in_=bass.AP(
                            tensor=b_dram.tensor,
                            offset=b_dram.offset + i * B,
                            ap=[[0, 128], [1, B]],
                        ),
                    )
                    for c in range(N_JCHUNK):
                        bias_ap = at[:, c, i : i + 1]
                        nc.vector.tensor_scalar(
                            out=vg[:, gi, c, :], in0=bb, scalar1=bias_ap,
                            scalar2=0.0, op0=Alu.add, op1=Alu.max,
                        )
                        nc.tensor.matmul(
                            psum_vs[32 * c : 32 * c + 1, :], ones_bf,
                            vg[:, gi, c, :],
                            start=(i == 0), stop=(i == PA - 1),
                            tile_position=(0, 32 * c),
                        )
                        n_pe_v += 1
                if g % SIGN_DEN < SIGN_NUM:
                    sg = scrp.tile([128, GROUP, N_JCHUNK, B], bf16, tag="sg")
                    nc.scalar.activation(
                        out=sg, in_=vg, func=Act.Sign, bias=epsb, scale=1.0,
                        accum_out=csA[:, n_a : n_a + 1],
                    )
                    n_a += 1
                else:
                    ind4 = scrp.tile([128, GROUP, N_JCHUNK, B], bf16, tag="sg")
                    nc.vector.tensor_scalar(
                        out=ind4, in0=vg, scalar1=EPS, scalar2=None, op0=Alu.is_gt,
                    )
                    for gi in range(GROUP):
                        for c in range(N_JCHUNK):
                            nc.tensor.matmul(
                                psum_cs[32 * c : 32 * c + 1, :], ones_bf,
                                ind4[:, gi, c, :],
                                start=(n_pe_c < N_JCHUNK),
                                stop=(n_pe_c >= GROUP * N_JCHUNK * (n_dvec - 1)),
                                tile_position=(0, 32 * c),
                            )
                            n_pe_c += 1

            # ---- final reduction -> out [128,2], out2 [8,B] ------------
            out_sb = singles.tile([128, 2], f32)
            nc.vector.reduce_sum(out_sb[:, 0:1], csA, axis=mybir.AxisListType.X)
            nc.vector.memset(out_sb[:, 1:2], 0.0)
            nc.sync.dma_start(out=out, in_=out_sb)
            acc_sb = singles.tile([128, 2, B], f32)
            for c in range(N_JCHUNK):
                r = 32 * c
                nc.vector.tensor_copy(acc_sb[r : r + 1, 0, :], psum_vs[r : r + 1, :])
                nc.scalar.copy(acc_sb[r : r + 1, 1, :], psum_cs[r : r + 1, :])
            nc.sync.dma_start(
                out=out2,
                in_=bass.AP(
                    tensor=acc_sb.tensor,
                    offset=acc_sb.offset,
                    ap=[[32, 4], [B, 2], [1, B]],
                ),
            )

    _split_sync_waits(nc)
    return nc


_PROGRAM = None


def _get_program():
    global _PROGRAM
    if _PROGRAM is None:
        _PROGRAM = _build_program()
    return _PROGRAM


def _make_in_maps(x, labels):
    x = np.ascontiguousarray(x, dtype=np.float32)
    labf = np.ascontiguousarray(labels.astype(np.float32))
    xT = np.ascontiguousarray(x.T)
    ident = np.eye(64, dtype=np.float32)
    in_maps = []
    for r in range(N_CORES):
        r0 = r * PA
        eyem = np.full((PA, B), BIG - MARGIN, dtype=np.float32)
        eyem[np.arange(PA), r0 + np.arange(PA)] += BIG
        in_maps.append(
            {
                "xT": xT,
                "labf": labf,
                "eyem": eyem,
                "ident": ident,
                "xT_ours": np.ascontiguousarray(xT[:, r0 : r0 + PA]),
                "lab_ours": np.ascontiguousarray(labf[r0 : r0 + PA]),
                "x_ours": np.ascontiguousarray(x[r0 : r0 + PA, :]),
            }
        )
    return in_maps


def run_device(x, labels, trace=False):
    """Run the SPMD program; returns (results, exec_time_ns)."""
    nc = _get_program()
    res = run_bass_kernel_spmd(
        nc, _make_in_maps(x, labels), core_ids=list(range(N_CORES)), trace=trace
    )
    return res


def host_reduce(res):
    n_groups = PA // GROUP
    n_sign = len([g for g in range(n_groups) if g % SIGN_DEN < SIGN_NUM])
    sign_elems = n_sign * 128 * GROUP * N_JCHUNK * B  # +/-1 terms per core
    S = 0.0
    C = 0.0
    for r in range(N_CORES):
        o = res.results[r]["out"].astype(np.float64)
        o2 = res.results[r]["out2"].astype(np.float64)
        S += o2[0::2].sum()
        C += o2[1::2].sum() + (o[:, 0].sum() + sign_elems) / 2.0
    return np.float32(S / (C + EPS))


def kernel(embeddings, labels):
    res = run_device(np.asarray(embeddings), np.asarray(labels), trace=False)
    return host_reduce(res)


# revision 51
# speedup vs baseline: 1.0203x; 1.0146x over previous
"""Batch-all triplet loss on 8 Trainium2 NeuronCores.

Math (reference): d = pairwise euclidean distances of embeddings [512, 256];
tl[i,j,k] = relu((d_ij - d_ik + margin) * mask); loss = sum(tl) / (count(tl > eps) + eps)
where mask requires labels[i]==labels[j], labels[i]!=labels[k], i!=j (j!=k, i!=k follow).

Device formulation: A[i,j] = d_ij + margin where (labels equal, i!=j) else -BIG;
B[i,k] = -d_ik where labels differ else -BIG.  Then per (i,j,k):
v = relu(A[i,j] + B[i,k]) equals the masked triplet loss exactly (masked
entries produce relu(very negative) = 0), and count = #{v > eps}.

Sharding: anchor axis i split 64-per-core across 8 cores.  Each core computes
its [64, 512, 512] block as 64 anchors x 4 j-chunks of [128 j, 512 k]:
  pass1 (VectorE, bf16 4x): v = max(B_bcast + A_col, 0) per chunk
  v row-sums: TensorE ones-matmuls accumulating into one PSUM bank
  count: per anchor, either one batched Sign activation over [128, 4*512]
         on ScalarE (count = (sum(sign(v - eps)) + N) / 2, fused accumulate)
         or a batched is_gt on VectorE + TensorE ones-matmuls (split tuned
         so ScalarE / VectorE / TensorE run near-equal busy time)
B_bcast rows are staged in DRAM and broadcast to 128 partitions via DMA.
Per-core partial sums are reduced on host: loss = S / (C + eps).
"""

import contextlib
import ctypes
import sys
import types

import numpy as np

import bass_rust
import concourse.bass as bass
import concourse.tile as tile
from concourse import mybir
from concourse.bass_utils import run_bass_kernel_spmd
from concourse.vector_clock import ScopedClock

# ---------------------------------------------------------------------------
# Environment shims (walrus drain-wait limit + NTFF profile hook under axon)
# ---------------------------------------------------------------------------

_MAX_WAITS_PER_INST = 1
_AXON_SO_PATH = "/opt/axon/libaxon_pjrt.so"


def _patched_drain_and_barrier(self, tick_clock, wait_clock):
    nc = self.nc
    drain_inst = nc.sync.drain()
    wait_clock.add_sem_waits(
        drain_inst.ins, ScopedClock({None: tick_clock.global_clock})
    )
    si = drain_inst.ins.sync_info
    if si is not None and si.on_wait and len(si.on_wait) > _MAX_WAITS_PER_INST:
        waits = list(si.on_wait)
        si.on_wait = waits[:_MAX_WAITS_PER_INST]
        rest = waits[_MAX_WAITS_PER_INST:]
        for i in range(0, len(rest), _MAX_WAITS_PER_INST):
            extra = nc.sync.drain()
            extra.ins.sync_info = bass_rust.SyncInfo(
                on_wait=rest[i : i + _MAX_WAITS_PER_INST], on_update=[]
            )

    nc.all_engine_barrier()
    assert self.sems is not None
    popped = nc._tile_sem_poison_stack.pop()
    assert popped is self._sem_poison
    nc.clear_and_free_semaphores(list(self.sems.allocated().values()))
    nc.all_engine_barrier()


def _ntff_profile_via_ctypes(so_path):
    try:
        lib = ctypes.CDLL(so_path)
    except OSError:
        return None
    if not hasattr(lib, "axon_start_nrt_profile"):
        return None
    lib.axon_start_nrt_profile.argtypes = [
        ctypes.POINTER(ctypes.c_int64),
        ctypes.c_size_t,
    ]
    lib.axon_start_nrt_profile.restype = ctypes.c_int64
    lib.axon_stop_nrt_profile.argtypes = [ctypes.c_char_p]
    lib.axon_stop_nrt_profile.restype = ctypes.c_int64

    @contextlib.contextmanager
    def _hook(output_dir, device_ids):
        import jax

        jax.devices()
        if device_ids:
            ids = (ctypes.c_int64 * len(device_ids))(*device_ids)
            rc = lib.axon_start_nrt_profile(ids, len(device_ids))
        else:
            rc = lib.axon_start_nrt_profile(None, 0)
        if rc != 0:
            raise RuntimeError(f"axon_start_nrt_profile rc={rc}")
        try:
            yield
        finally:
            n = lib.axon_stop_nrt_profile(str(output_dir).encode())
            if n < 0:
                raise RuntimeError(f"axon_stop_nrt_profile rc={n}")
            if n == 0:
                print(f"profile: ZERO files written to {output_dir}", file=sys.stderr)

    return _hook


def _split_sync_waits(nc, max_waits=1):
    """This toolchain's walrus rejects instructions carrying more than one
    semaphore wait.  Hoist extra waits onto standalone EventSemaphore
    instructions inserted just before the owner (same engine, so sequencer
    program order preserves the blocking semantics)."""
    n_split = 0
    for f in nc.m.functions:
        for blk in f.blocks:
            insts = list(blk.instructions)
            out = []
            changed = False
            for inst in insts:
                si = getattr(inst, "sync_info", None)
                if si is not None and si.on_wait and len(si.on_wait) > max_waits:
                    waits = list(si.on_wait)
                    extra, keep = waits[:-max_waits], waits[-max_waits:]
                    for j, w in enumerate(extra):
                        ev = mybir.InstEventSemaphore(name=f"{inst.name}-sw{j}", ins=[], outs=[])
                        ev.name = f"{inst.name}-sw{j}"
                        ev.engine = inst.engine
                        ev.sync_info = bass_rust.SyncInfo(on_wait=[w], on_update=[])
                        out.append(ev)
                        n_split += 1
                    si.on_wait = keep
                    changed = True
                out.append(inst)
            if changed:
                blk.instructions = out
    return n_split


def _install_shims():
    tile.TileContext._drain_and_barrier = _patched_drain_and_barrier
    if "antenv.axon_hooks" not in sys.modules:
        m = types.ModuleType("antenv.axon_hooks")
        _state = {"hook": _ntff_profile_via_ctypes(_AXON_SO_PATH)}
        m.set_axon_ntff_profile_hook = lambda hook: _state.__setitem__("hook", hook)
        m.get_axon_ntff_profile_hook = lambda: _state["hook"]
        sys.modules["antenv.axon_hooks"] = m


_install_shims()

# ---------------------------------------------------------------------------
# Problem constants (hardcoded per spec)
# ---------------------------------------------------------------------------

B = 512          # batch (anchors)
D = 256          # embedding dim
N_CORES = 8
PA = B // N_CORES  # anchors per core = 64
EPS = 1e-8
MARGIN = 1.0
BIG = 50000.0
N_JCHUNK = B // 128  # 4 j-chunks of 128 partitions
# count engine assignment over anchor PAIRS: SIGN_NUM of every SIGN_DEN pairs
# count via one batched Sign activation on ScalarE (over both anchors' v
# tiles); the rest via batched is_gt on VectorE plus TensorE ones-matmuls.
SIGN_NUM = 3
SIGN_DEN = 5
GROUP = 1  # anchors per count group
DSHIFT = 0.5  # added under the sqrt; keeps it clamp-free, cancels in d_ij-d_ik

f32 = mybir.dt.float32
bf16 = mybir.dt.bfloat16
Alu = mybir.AluOpType
Act = mybir.ActivationFunctionType


def _build_program():
    nc = bass.Bass("TRN2", target_bir_lowering=False, debug=False)

    xT = nc.dram_tensor("xT", [D, B], f32, kind="ExternalInput").ap()
    labf = nc.dram_tensor("labf", [B], f32, kind="ExternalInput").ap()
    eyem = nc.dram_tensor("eyem", [PA, B], f32, kind="ExternalInput").ap()
    ident = nc.dram_tensor("ident", [64, 64], f32, kind="ExternalInput").ap()
    out = nc.dram_tensor("out", [128, 2], f32, kind="ExternalOutput").ap()
    out2 = nc.dram_tensor("out2", [1, 2 * B], f32, kind="ExternalOutput").ap()

    b_dram = nc.dram_tensor("b_scratch", [PA, B], bf16, kind="Internal").ap()

    # The per-core anchor window is expressed through input VALUES (one SPMD
    # program): the host passes core-sliced tensors directly.
    xTo = nc.dram_tensor("xT_ours", [D, PA], f32, kind="ExternalInput").ap()
    labo = nc.dram_tensor("lab_ours", [PA], f32, kind="ExternalInput").ap()
    xo_in = nc.dram_tensor("x_ours", [PA, D], f32, kind="ExternalInput").ap()

    with tile.TileContext(nc) as tc:
        with contextlib.ExitStack() as ctx:
            singles = ctx.enter_context(tc.tile_pool(name="singles", bufs=1))
            prol = ctx.enter_context(tc.tile_pool(name="prol", bufs=2))
            psum_p = ctx.enter_context(tc.tile_pool(name="psum_p", bufs=1, space="PSUM"))
            psum_t = ctx.enter_context(tc.tile_pool(name="psum_t", bufs=2, space="PSUM"))
            bcast = ctx.enter_context(tc.tile_pool(name="bcast", bufs=6))
            vpool = ctx.enter_context(tc.tile_pool(name="vpool", bufs=4))
            scrp = ctx.enter_context(tc.tile_pool(name="scrp", bufs=3))

            # ---- loads (spread across two DMA paths) -------------------
            xT_t = singles.tile([128, 2, B], f32)          # xT[(c p), k] -> [p, c, k]
            nc.sync.dma_start(out=xT_t, in_=xT.rearrange("(c p) k -> p c k", p=128))
            xTo_t = singles.tile([128, 2, PA], f32)        # our anchor columns
            nc.gpsimd.dma_start(out=xTo_t, in_=xTo.rearrange("(c p) k -> p c k", p=128))
            labo_t = singles.tile([PA, 1], f32)
            nc.gpsimd.dma_start(
                out=labo_t,
                in_=bass.AP(tensor=labo.tensor, offset=labo.offset, ap=[[1, PA], [1, 1]]),
            )
            lab_bcast = singles.tile([PA, B], f32)
            nc.gpsimd.dma_start(
                out=lab_bcast,
                in_=bass.AP(tensor=labf.tensor, offset=labf.offset, ap=[[0, PA]] + list(labf.ap)),
            )
            eyem_t = singles.tile([PA, B], f32)
            nc.gpsimd.dma_start(out=eyem_t, in_=eyem)
            ident_t = singles.tile([64, 64], f32)
            nc.gpsimd.dma_start(out=ident_t, in_=ident)
            xo_t = singles.tile([PA, D], f32)
            nc.sync.dma_start(out=xo_t, in_=xo_in)

            ones_bf = singles.tile([128, 1], bf16)
            nc.vector.memset(ones_bf, 1.0)
            epsb = singles.tile([128, 1], f32)
            nc.vector.memset(epsb, -EPS)

            # ---- sq_k = sum_d x^2 (PE path: square then ones-matmul) ---
            xsq = singles.tile([128, 2, B], bf16)
            nc.vector.scalar_tensor_tensor(
                out=xsq, in0=xT_t, scalar=1.0, in1=xT_t,
                op0=Alu.mult, op1=Alu.mult,
            )
            psum_sq = psum_p.tile([1, B], f32, tag="psum_sq")
            nc.tensor.matmul(psum_sq, ones_bf, xsq[:, 0, :], start=True, stop=False)
            nc.tensor.matmul(psum_sq, ones_bf, xsq[:, 1, :], start=False, stop=True)
            sq_row = singles.tile([1, B], f32)
            nc.vector.tensor_copy(sq_row, psum_sq)
            # our anchors' sq as a [PA, 1] per-partition scalar (from x_ours)
            sqo_t = singles.tile([PA, 1], f32)
            sqo_scratch = prol.tile([PA, D], f32, tag="sqo_scratch")
            nc.vector.scalar_tensor_tensor(
                out=sqo_scratch, in0=xo_t, scalar=1.0, in1=xo_t,
                op0=Alu.mult, op1=Alu.mult, accum_out=sqo_t,
            )
            # +DSHIFT so sqrt's argument stays positive without a clamp op
            # (fp residue on the diagonal can be slightly negative).  The
            # distance shift cancels to first order in d_ij - d_ik.
            sqo1_t = singles.tile([PA, 1], f32)
            nc.vector.tensor_scalar(
                out=sqo1_t, in0=sqo_t, scalar1=DSHIFT, scalar2=None, op0=Alu.add,
            )

            # ---- G slab: psum_d = -2 * x_ours @ x^T + sq_k -------------
            m2xT = singles.tile([128, 2, PA], f32)
            for c in range(2):
                nc.vector.tensor_scalar(
                    out=m2xT[:, c, :], in0=xTo_t[:, c, :],
                    scalar1=-2.0, scalar2=None, op0=Alu.mult,
                )
            ones1 = singles.tile([1, PA], f32)
            nc.vector.memset(ones1, 1.0)

            psum_d = psum_p.tile([PA, B], f32, tag="psum_d")
            nc.tensor.matmul(psum_d, m2xT[:, 0, :], xT_t[:, 0, :], start=True, stop=False)
            nc.tensor.matmul(psum_d, m2xT[:, 1, :], xT_t[:, 1, :], start=False, stop=False)
            nc.tensor.matmul(psum_d, ones1, sq_row, start=False, stop=True)

            # ---- distances: d = sqrt(psum + sq_i + DSHIFT) -------------
            dslab = singles.tile([PA, B], f32)
            nc.scalar.activation(
                out=dslab, in_=psum_d, func=Act.Sqrt, bias=sqo1_t, scale=1.0,
            )

            # ---- masks and A/B matrices --------------------------------
            leq = prol.tile([PA, B], f32, tag="leq")
            nc.vector.tensor_scalar(
                out=leq, in0=lab_bcast, scalar1=labo_t, scalar2=None, op0=Alu.is_equal,
            )
            a32 = singles.tile([PA, B], f32)
            nc.vector.scalar_tensor_tensor(
                out=a32, in0=leq, scalar=BIG, in1=eyem_t,
                op0=Alu.mult, op1=Alu.subtract,
            )
            nc.vector.tensor_tensor(out=a32, in0=a32, in1=dslab, op=Alu.add)
            bbf = singles.tile([PA, B], bf16)
            nc.vector.scalar_tensor_tensor(
                out=bbf, in0=leq, scalar=-BIG, in1=dslab,
                op0=Alu.mult, op1=Alu.subtract,
            )
            nc.sync.dma_start(out=b_dram, in_=bbf)

            # ---- A transposed: AT[j, c, i] = A[i, c*128 + j] ------------
            at = singles.tile([128, N_JCHUNK, PA], f32)
            for c in range(N_JCHUNK):
                pt = psum_t.tile([128, PA], f32, tag="pt")
                nc.tensor.transpose(pt, a32[:, c * 128 : (c + 1) * 128], ident_t)
                nc.vector.tensor_copy(at[:, c, :], pt)

            # ---- accumulators ------------------------------------------
            n_tiles = PA * N_JCHUNK
            n_groups = PA // GROUP
            sign_groups = [g for g in range(n_groups) if g % SIGN_DEN < SIGN_NUM]
            n_sign = len(sign_groups)
            n_dvec = n_groups - n_sign
            csA = singles.tile([128, max(n_sign, 1)], f32)
            psum_vs = psum_p.tile([1, B], f32, tag="psum_vs")
            psum_cs = psum_p.tile([1, B], f32, tag="psum_cs")

            # ---- main loop ---------------------------------------------
            # pass1: all chunks on DVE (fused add+relu, bf16 4x) into one
            # per-group [128, GROUP*4, B] tile; v row-sums via TensorE
            # ones-matmuls.  count: per anchor-group either one batched Sign
            # activation on ScalarE (count = (sum(sign(v-eps)) + N)/2) or a
            # batched is_gt on DVE plus TensorE ones-matmuls.
            n_pe_v = 0
            n_pe_c = 0
            n_a = 0
            for g in range(n_groups):
                vg = vpool.tile([128, GROUP, N_JCHUNK, B], bf16, tag="vg")
                for gi in range(GROUP):
                    i = g * GROUP + gi
                    bb = bcast.tile([128, B], bf16, tag="bb")
                    nc.sync.dma_start(
                        out=bb,
                        in_=bass.AP(
                            tensor=b_dram.tensor,
                            offset=b_dram.offset + i * B,
                            ap=[[0, 128], [1, B]],
                        ),
                    )
                    for c in range(N_JCHUNK):
                        bias_ap = at[:, c, i : i + 1]
                        nc.vector.tensor_scalar(
                            out=vg[:, gi, c, :], in0=bb, scalar1=bias_ap,
                            scalar2=0.0, op0=Alu.add, op1=Alu.max,
                        )
                        nc.tensor.matmul(
                            psum_vs, ones_bf, vg[:, gi, c, :],
                            start=(n_pe_v == 0), stop=(n_pe_v == n_tiles - 1),
                        )
                        n_pe_v += 1
                if g % SIGN_DEN < SIGN_NUM:
                    sg = scrp.tile([128, GROUP, N_JCHUNK, B], bf16, tag="sg")
                    nc.scalar.activation(
                        out=sg, in_=vg, func=Act.Sign, bias=epsb, scale=1.0,
                        accum_out=csA[:, n_a : n_a + 1],
                    )
                    n_a += 1
                else:
                    ind4 = scrp.tile([128, GROUP, N_JCHUNK, B], bf16, tag="sg")
                    nc.vector.tensor_scalar(
                        out=ind4, in0=vg, scalar1=EPS, scalar2=None, op0=Alu.is_gt,
                    )
                    for gi in range(GROUP):
                        for c in range(N_JCHUNK):
                            nc.tensor.matmul(
                                psum_cs, ones_bf, ind4[:, gi, c, :],
                                start=(n_pe_c == 0),
                                stop=(n_pe_c == GROUP * N_JCHUNK * n_dvec - 1),
                            )
                            n_pe_c += 1

            # ---- final reduction -> out [128,2], out2 [1,2B] -----------
            out_sb = singles.tile([128, 2], f32)
            nc.vector.reduce_sum(out_sb[:, 0:1], csA, axis=mybir.AxisListType.X)
            nc.vector.memset(out_sb[:, 1:2], 0.0)
            nc.sync.dma_start(out=out, in_=out_sb)
            vs_row = singles.tile([1, 2 * B], f32)
            nc.vector.tensor_copy(vs_row[:, 0:B], psum_vs)
            nc.scalar.copy(vs_row[:, B : 2 * B], psum_cs)
            nc.sync.dma_start(out=out2, in_=vs_row)

    _split_sync_waits(nc)
    return nc


_PROGRAM = None


def _get_program():
    global _PROGRAM
    if _PROGRAM is None:
        _PROGRAM = _build_program()
    return _PROGRAM


def _make_in_maps(x, labels):
    x = np.ascontiguousarray(x, dtype=np.float32)
    labf = np.ascontiguousarray(labels.astype(np.float32))
    xT = np.ascontiguousarray(x.T)
    ident = np.eye(64, dtype=np.float32)
    in_maps = []
    for r in range(N_CORES):
        r0 = r * PA
        eyem = np.full((PA, B), BIG - MARGIN, dtype=np.float32)
        eyem[np.arange(PA), r0 + np.arange(PA)] += BIG
        in_maps.append(
            {
                "xT": xT,
                "labf": labf,
                "eyem": eyem,
                "ident": ident,
                "xT_ours": np.ascontiguousarray(xT[:, r0 : r0 + PA]),
                "lab_ours": np.ascontiguousarray(labf[r0 : r0 + PA]),
                "x_ours": np.ascontiguousarray(x[r0 : r0 + PA, :]),
            }
        )
    return in_maps


def run_device(x, labels, trace=False):
    """Run the SPMD program; returns (results, exec_time_ns)."""
    nc = _get_program()
    res = run_bass_kernel_spmd(
        nc, _make_in_maps(x, labels), core_ids=list(range(N_CORES)), trace=trace
    )
    return res


def host_reduce(res):
    n_groups = PA // GROUP
    n_sign = len([g for g in range(n_groups) if g % SIGN_DEN < SIGN_NUM])
    sign_elems = n_sign * 128 * GROUP * N_JCHUNK * B  # +/-1 terms per core
    S = 0.0
    C = 0.0
    for r in range(N_CORES):
        o = res.results[r]["out"].astype(np.float64)
        o2 = res.results[r]["out2"].astype(np.float64).reshape(2, B)
        S += o2[0].sum()
        C += o2[1].sum() + (o[:, 0].sum() + sign_elems) / 2.0
    return np.float32(S / (C + EPS))


def kernel(embeddings, labels):
    res = run_device(np.asarray(embeddings), np.asarray(labels), trace=False)
    return host_reduce(res)
